# revision 1
# baseline (speedup 1.0000x reference)
"""Trainium2 Bass kernel for nn_CrossAttention (B=2, N=M=4096, Dq=512, Dc=768,
8 heads x 64).

Sharding (8 cores): core c handles batch b = c//4 and heads {2*(c%4), 2*(c%4)+1}.
Each core computes its two heads' attention over the full sequence and the
partial output projection (its 128 rows of Wo). Host sums the 4 partials per
batch and adds the output bias.

Per-core kernel layout strategy (everything "transposed" so contraction dims
sit on SBUF partitions; all matmuls fp32r = full-rate PE):
  xT/ctxT via PE transposes of natural tiles (PSUM -> SBUF evictions on DVE)
  qT [128(2 heads*64), n] = Wq2^T @ xT ;  kT [128, m] = (Wk2*scale)^T @ ctxT
  vab [m, 2, 65] = [v_h | 1]  (appended ones column => sum(exp) for free)
  scoresT [m, n0] = kT_h^T @ qT_h  (two heads as K=64 row-group pairs, psum
    tile [128, 1024] = both heads of one m-chunk, double buffered)
  expT = Exp(scoresT) on ACT (the throughput-critical engine), one
    [128, 1024] instruction per m-chunk covering both heads
  avT [65, n0] += [v_h | 1]^T @ expT  (psum-accumulated; row 64 = sum of exp)
  final_h [n, 512] = avT_h^T @ Wo_ext_h ; sumexp column via N=2 matmul
    against [0..0,1]^T ;  out = final_h0/s_h0 + final_h1/s_h1 (per-partition
    scalar multiplies on DVE), DMA to DRAM.
The projection pipeline (ctx/x loads, transposes, q/k/v matmuls) is fully
interleaved into the attention m/n loops; PSUM (8 banks) is shared via three
pool tags: sc (2x2 banks: score tiles + transpose scratch), av (2: running
AV accumulators + sumexp transpose), aux (2: k/v/q projection psums + final
output projection tiles).
"""

import numpy as np

HEADS = 8
D = 64
B = 2
N = 4096
DQ = 512
M = 4096
DC = 768
SCALE = D ** -0.5
NCORES = 8

_CACHE = {}


def _build_module(n_iters=1, act_evicts=False):
    import concourse.bacc as bacc
    import concourse.mybir as mybir
    import concourse.tile as tile
    from concourse.masks import make_identity
    from contextlib import ExitStack

    FP32 = mybir.dt.float32
    R = mybir.dt.float32r
    EXP = mybir.ActivationFunctionType.Exp

    nc = bacc.Bacc("TRN2", target_bir_lowering=False, debug=False)

    xb = nc.dram_tensor("xb", [N, DQ], FP32, kind="ExternalInput")
    cb = nc.dram_tensor("cb", [M, DC], FP32, kind="ExternalInput")
    wq = nc.dram_tensor("wq", [DQ, 128], FP32, kind="ExternalInput")
    wk = nc.dram_tensor("wk", [DC, 128], FP32, kind="ExternalInput")
    wv = nc.dram_tensor("wv", [DC, 128], FP32, kind="ExternalInput")
    wo = nc.dram_tensor("wo", [128, DQ], FP32, kind="ExternalInput")
    y = nc.dram_tensor("y", [N, DQ], FP32, kind="ExternalOutput")

    with tile.TileContext(nc) as tc, ExitStack() as top:
        const = top.enter_context(tc.tile_pool(name="const", bufs=1))
        ident = const.tile([128, 128], FP32)
        make_identity(nc, ident)

        # weights: DMA into fp32 staging, convert (round) into fp32r tiles
        wstage = const.tile([128, 16, 128], FP32)
        for d in range(4):
            nc.gpsimd.dma_start(wstage[:, d, :], wq[d * 128:(d + 1) * 128, :])
        for d in range(6):
            nc.gpsimd.dma_start(wstage[:, 4 + d, :], wk[d * 128:(d + 1) * 128, :])
            nc.gpsimd.dma_start(wstage[:, 10 + d, :], wv[d * 128:(d + 1) * 128, :])
        wq_sb = const.tile([128, 4, 128], R)
        wk_sb = const.tile([128, 6, 128], R)
        wv_sb = const.tile([128, 6, 128], R)
        nc.vector.tensor_copy(wq_sb[:, :, :], wstage[:, 0:4, :])
        nc.vector.tensor_copy(wk_sb[:, :, :], wstage[:, 4:10, :])
        nc.vector.tensor_copy(wv_sb[:, :, :], wstage[:, 10:16, :])
        # Wo extended: [65, 2, 512]; row 64 zero
        wos = const.tile([65, 2, 512], FP32)
        nc.gpsimd.dma_start(wos[0:64, 0, :], wo[0:64, :])
        nc.gpsimd.dma_start(wos[0:64, 1, :], wo[64:128, :])
        nc.vector.memset(wos[64:65, :, :], 0.0)
        woe = const.tile([65, 2, 512], R)
        nc.vector.tensor_copy(woe[:, :, :], wos[:, :, :])
        # s-extraction columns (N=2 for fp32r evenness): rows 0-63 zero, row 64 one
        scst = const.tile([65, 2], FP32)
        nc.vector.memset(scst[:, :], 0.0)
        nc.vector.memset(scst[64:65, :], 1.0)
        scol = const.tile([65, 2], R)
        nc.vector.tensor_copy(scol[:, :], scst[:, :])
        # staged ones for vab's appended-one columns
        onesst = const.tile([128, 32, 2, 1], FP32)
        nc.vector.memset(onesst[:, :, :, :], 1.0)

        res = top.enter_context(tc.tile_pool(name="res", bufs=1))

        import concourse.mybir as _mb
        ld = top.enter_context(tc.tile_pool(name="ld", bufs=6))
        tsb = top.enter_context(tc.tile_pool(name="tsb", bufs=3))
        vsb = top.enter_context(tc.tile_pool(name="vsb", bufs=2))
        scp = top.enter_context(tc.tile_pool(name="scp", bufs=2, space="PSUM"))
        avp = top.enter_context(tc.tile_pool(name="avp", bufs=2, space="PSUM"))
        auxp = top.enter_context(tc.tile_pool(name="auxp", bufs=2, space="PSUM"))
        exp = top.enter_context(tc.tile_pool(name="exp", bufs=10))
        avs = top.enter_context(tc.tile_pool(name="avs", bufs=6))
        osb = top.enter_context(tc.tile_pool(name="osb", bufs=4))
        rsb = top.enter_context(tc.tile_pool(name="rsb", bufs=2))

        for _it in range(n_iters):
            # residents for this iteration
            qT = res.tile([128, N], R, tag="qT")
            kT = res.tile([128, M], R, tag="kT")
            vab = res.tile([128, 32, 2, 65], R, tag="vab")
            nc.vector.tensor_copy(vab[:, :, :, 64:65], onesst[:, :, :, :])

            def x_chunk(n0, early=False):
                """x rows [n0*512, +512) -> transposed xT scratch -> qT cols."""
                xts = tsb.tile([128, 4, 512], R, tag="xt", bufs=2, name="xts")
                for s in range(4):
                    xn = ld.tile([128, 512], FP32, tag="xn", name="xn")
                    nc.scalar.dma_start(
                        xn[:], xb[(n0 * 4 + s) * 128:(n0 * 4 + s + 1) * 128, :])
                    tpx = scp.tile([128, 1024], FP32, tag="sc", name="tpx")
                    for d in range(4):
                        nc.tensor.transpose(
                            tpx[:, d * 128:(d + 1) * 128],
                            xn[:, d * 128:(d + 1) * 128], ident)
                    eng = nc.scalar.copy if (act_evicts and early and s % 2 == 0) \
                        else nc.vector.tensor_copy
                    eng(xts[:, :, s * 128:(s + 1) * 128],
                        tpx[:, 0:512].rearrange("p (i c) -> p i c", i=4))
                qp = auxp.tile([128, 512], FP32, tag="aux", name="qp")
                for d in range(4):
                    nc.tensor.matmul(
                        qp[:], wq_sb[:, d, :], xts[:, d, :],
                        start=(d == 0), stop=(d == 3))
                nc.vector.tensor_copy(qT[:, n0 * 512:(n0 + 1) * 512], qp[:])

            def ctx_chunk(m0):
                """context rows [m0*512, +512) -> kT cols + vab tiles."""
                cts = tsb.tile([128, 6, 512], R, tag="ct", bufs=2, name="cts")
                for s in range(4):
                    cn = ld.tile([128, 768], FP32, tag="cn", name="cn")
                    nc.sync.dma_start(
                        cn[:], cb[(m0 * 4 + s) * 128:(m0 * 4 + s + 1) * 128, :])
                    for dpair in range(2):
                        tpp = scp.tile([128, 1024], FP32, tag="sc", name="tpp")
                        for i in range(3):
                            d = dpair * 3 + i
                            nc.tensor.transpose(
                                tpp[:, i * 128:(i + 1) * 128],
                                cn[:, d * 128:(d + 1) * 128], ident)
                        eng = nc.scalar.copy if (act_evicts and dpair == 0) \
                            else nc.vector.tensor_copy
                        eng(cts[:, dpair * 3:(dpair + 1) * 3,
                                s * 128:(s + 1) * 128],
                            tpp[:, 0:384].rearrange("p (i c) -> p i c", i=3))
                kp = auxp.tile([128, 512], FP32, tag="aux", name="kp")
                for d in range(6):
                    nc.tensor.matmul(
                        kp[:], wk_sb[:, d, :], cts[:, d, :],
                        start=(d == 0), stop=(d == 5))
                (nc.scalar.copy if act_evicts else nc.vector.tensor_copy)(kT[:, m0 * 512:(m0 + 1) * 512], kp[:])
                vp = auxp.tile([128, 512], FP32, tag="aux", name="vp")
                for d in range(6):
                    nc.tensor.matmul(
                        vp[:], wv_sb[:, d, :], cts[:, d, :],
                        start=(d == 0), stop=(d == 5))
                vt = vsb.tile([128, 512], FP32, tag="vt", name="vt")
                (nc.scalar.copy if act_evicts else nc.vector.tensor_copy)(vt[:], vp[:])
                v2 = auxp.tile([128, 512], FP32, tag="aux", name="v2")
                for s in range(4):
                    nc.tensor.transpose(
                        v2[:, s * 128:(s + 1) * 128],
                        vt[:, s * 128:(s + 1) * 128], ident)
                for s in range(4):
                    mc = m0 * 4 + s
                    nc.vector.tensor_copy(
                        vab[:, mc, :, 0:64],
                        v2[:, s * 128:(s + 1) * 128].rearrange(
                            "p (h c) -> p h c", h=2))

            def att_mc(n0, mc, av0, av1):
                nsl = slice(n0 * 512, (n0 + 1) * 512)
                msl = slice(mc * 128, (mc + 1) * 128)
                sc = scp.tile([128, 1024], FP32, tag="sc", name="sc")
                nc.tensor.matmul(sc[:, 0:512], kT[0:64, msl], qT[0:64, nsl])
                nc.tensor.matmul(sc[:, 512:1024], kT[64:128, msl],
                                 qT[64:128, nsl])
                ex = exp.tile([128, 1024], R, tag="ex", name="ex")
                nc.scalar.activation(ex[:], sc[:], EXP)
                nc.tensor.matmul(av0[:], vab[:, mc, 0, :], ex[:, 0:512],
                                 start=(mc == 0), stop=(mc == 31))
                nc.tensor.matmul(av1[:], vab[:, mc, 1, :], ex[:, 512:1024],
                                 start=(mc == 0), stop=(mc == 31))

            def wo_stage(n0, av0, av1):
                a0 = avs.tile([65, 512], R, tag="as", name="a0")
                a1 = avs.tile([65, 512], R, tag="as", name="a1")
                nc.vector.tensor_copy(a0[:], av0[:])
                nc.vector.tensor_copy(a1[:], av1[:])
                sps = avp.tile([128, 16], FP32, tag="av", name="sps")
                for sub in range(4):
                    ssl = slice(sub * 128, (sub + 1) * 128)
                    nc.tensor.matmul(
                        sps[:, 2 * sub:2 * sub + 2], a0[:, ssl], scol[:])
                    nc.tensor.matmul(
                        sps[:, 8 + 2 * sub:10 + 2 * sub], a1[:, ssl], scol[:])
                rr = rsb.tile([128, 16], FP32, tag="rr", name="rr")
                nc.vector.reciprocal(rr[:], sps[:])
                for sub in range(4):
                    ssl = slice(sub * 128, (sub + 1) * 128)
                    f0 = auxp.tile([128, 512], FP32, tag="aux", name="f0")
                    f1 = auxp.tile([128, 512], FP32, tag="aux", name="f1")
                    nc.tensor.matmul(f0[:], a0[:, ssl], woe[:, 0, :])
                    nc.tensor.matmul(f1[:], a1[:, ssl], woe[:, 1, :])
                    ot = osb.tile([128, 512], FP32, tag="ot", name="ot")
                    nc.vector.tensor_scalar_mul(
                        ot[:], f0[:], rr[:, 2 * sub:2 * sub + 1])
                    nc.vector.scalar_tensor_tensor(
                        ot[:], f1[:], rr[:, 8 + 2 * sub:9 + 2 * sub], ot[:],
                        op0=_mb.AluOpType.mult, op1=_mb.AluOpType.add)
                    nc.sync.dma_start(
                        y[(n0 * 4 + sub) * 128:(n0 * 4 + sub + 1) * 128, :],
                        ot[:])

            # fully interleaved schedule: ctx/x projection work rides along
            # with the attention m-loop; no phase barrier.
            x_chunk(0, early=True)
            for n0 in range(8):
                av0 = avp.tile([65, 512], FP32, tag="av", name="av0")
                av1 = avp.tile([65, 512], FP32, tag="av", name="av1")
                if n0 == 0:
                    for m0 in range(8):
                        ctx_chunk(m0)
                        if m0 == 2:
                            x_chunk(1, early=True)
                        for s4 in range(4):
                            att_mc(0, m0 * 4 + s4, av0, av1)
                else:
                    for mc in range(32):
                        att_mc(n0, mc, av0, av1)
                        if mc == 8 and n0 < 7:
                            x_chunk(n0 + 1)
                wo_stage(n0, av0, av1)

    nc.compile()
    return nc


def _get_module(n_iters=1, act_evicts=False):
    key = (n_iters, act_evicts)
    if key not in _CACHE:
        _CACHE[key] = _build_module(n_iters, act_evicts)
    return _CACHE[key]


def _make_in_maps(x, context, Wq, Wk, Wv, Wo):
    in_maps = []
    for c in range(NCORES):
        b = c // 4
        h0 = 2 * (c % 4)
        cols = slice(h0 * D, (h0 + 2) * D)
        in_maps.append({
            "xb": np.ascontiguousarray(x[b]),
            "cb": np.ascontiguousarray(context[b]),
            "wq": np.ascontiguousarray(Wq[:, cols]),
            "wk": np.ascontiguousarray(Wk[:, cols] * SCALE),
            "wv": np.ascontiguousarray(Wv[:, cols]),
            "wo": np.ascontiguousarray(Wo[cols, :]),
        })
    return in_maps


def run_sharded(x, context, Wq, Wk, Wv, Wo, bo, n_iters=1):
    from concourse.bass_utils import run_bass_kernel_spmd
    nc = _get_module(n_iters)
    in_maps = _make_in_maps(x, context, Wq, Wk, Wv, Wo)
    res = run_bass_kernel_spmd(nc, in_maps, core_ids=list(range(NCORES)))
    ys = [res.results[c]["y"] for c in range(NCORES)]
    out = np.empty((B, N, DQ), np.float32)
    for b in range(B):
        acc = ys[4 * b].astype(np.float32).copy()
        for c in range(4 * b + 1, 4 * b + 4):
            acc += ys[c]
        out[b] = acc + bo[None, :]
    return out


def kernel(**inputs):
    x = np.asarray(inputs["x"], np.float32)
    context = np.asarray(inputs["context"], np.float32)
    Wq = np.asarray(inputs["Wq"], np.float32)
    Wk = np.asarray(inputs["Wk"], np.float32)
    Wv = np.asarray(inputs["Wv"], np.float32)
    Wo = np.asarray(inputs["Wo"], np.float32)
    bo = np.asarray(inputs["bo"], np.float32)
    return run_sharded(x, context, Wq, Wk, Wv, Wo, bo, n_iters=1)

